# revision 2
# baseline (speedup 1.0000x reference)
"""Trainium2 Bass kernel for nn_DiffusionGraphConv_89936615178296.

out = relu(A_hat @ (x @ (W1+W2)) + b1 + b2), A_hat = D^-1/2 (A+I) D^-1/2.

Reformulation: out = relu(dinv * (Agg(dinv*x) @ W) + b), W = W1+W2,
dinv = rsqrt(1 + in_degree); the per-edge norm dinv[src]*dinv[dst] is
separable so per-edge work is a pure gather + segment-sum of x' = dinv*x.

8 cores, no collectives. Nodes sharded round-robin in degree-sorted order
(uniform slot schedule across cores, ~13% padding). Each core:
  P0: builds the fp16 gather table x' = dinv*x for the full graph in its
      local HBM (partition-contiguous strips).
  P1: per destination tile group, gathers source rows into a [128, nt, K,
      32] fp16 slot grid (one slot column per indirect DMA; each column
      supplies one dynamic row offset per partition), pairwise-tree
      reduces on the vector engine, applies dinv, multiplies by W on the
      tensor engine (batched transpose + block-diag matmul), relu, and
      accumulates the output shard in SBUF; one final DMA writes it out.

Host does integer index prep only (degree counts, shard permutation, CSR
slot layout); all f32 arithmetic runs on device.
"""

import numpy as np
from contextlib import ExitStack

import concourse.bass as bass
import concourse.bacc as bacc
import concourse.tile as tile
from concourse import mybir
from concourse.masks import make_identity
from concourse.bass_utils import run_bass_kernel_spmd

N, E, C = 100000, 1600000, 32
M = 8
P = 128
TILES = 98                  # 98*128 = 12544 rows per core >= 12500
ROWS_PER_CORE = TILES * P
TABLE_STRIPS = 783          # 783*128 = 100224 >= N+1 (zero row at index N)
TABLE_ROWS = TABLE_STRIPS * P
ZERO_ROW = N
G_STRIP = 64                # strip width (table rows per partition per strip)
NT = 4                      # tiles per group


def _host_prep(x, edge_index):
    src = np.asarray(edge_index[0], dtype=np.int64)
    dst = np.asarray(edge_index[1], dtype=np.int64)
    x = np.asarray(x, dtype=np.float32)

    deg = np.bincount(dst, minlength=N).astype(np.int64) + 1

    order = np.argsort(-deg, kind="stable")
    ar = np.arange(N)
    core_of = np.empty(N, np.int64)
    pos_of = np.empty(N, np.int64)
    core_of[order] = ar % M
    pos_of[order] = ar // M

    degs_sorted = deg[order]
    Kt = [int(degs_sorted[t * P * M]) if t * P * M < N else 1 for t in range(TILES)]

    # groups of NT tiles, uniform K per group, rounded to multiple of 4
    groups = []  # (t0, t1, Kg, slot_col_offset)
    col = 0
    for g0 in range(0, TILES, NT):
        g1 = min(g0 + NT, TILES)
        Kg = max(Kt[g0:g1])
        Kg = 4 * ((Kg + 3) // 4)
        groups.append((g0, g1, Kg, col))
        col += (g1 - g0) * Kg
    SCOLS = col  # total slot columns per core

    # CSR by destination
    eorder = np.argsort(dst, kind="stable")
    src_sorted = src[eorder].astype(np.int32)
    starts = np.concatenate([[0], np.cumsum(deg - 1)])

    t_of = pos_of >> 7
    p_of = pos_of & 127

    # column index of tile t's slot k
    colbase = np.empty(TILES, np.int64)
    Kg_of = np.empty(TILES, np.int64)
    for (g0, g1, Kg, c0) in groups:
        for t in range(g0, g1):
            colbase[t] = c0 + (t - g0) * Kg
            Kg_of[t] = Kg

    idx_all = np.full((M, P, SCOLS), ZERO_ROW, np.int32)
    # self-loop at slot 0
    idx_all[core_of, p_of, colbase[t_of]] = np.arange(N, dtype=np.int32)
    # edges at slots 1..deg-1
    dstn = dst[eorder]
    rank = np.arange(E, dtype=np.int64) - starts[dstn]
    cols = colbase[t_of[dstn]] + 1 + rank
    idx_all[core_of[dstn], p_of[dstn], cols] = src_sorted

    deg_pad = np.ones(TABLE_ROWS, np.float32)
    deg_pad[:N] = deg
    # strip layout: strip s covers rows [s*128*G, ...), partition p gets rows
    # base + p*Gs + g (contiguous per partition)
    deg_arr = np.ones((P, TABLE_STRIPS), np.float32)
    colpos = 0
    for s0 in range(0, TABLE_STRIPS, G_STRIP):
        g = min(G_STRIP, TABLE_STRIPS - s0)
        blk = deg_pad[s0 * P : (s0 + g) * P].reshape(P, g)
        deg_arr[:, colpos : colpos + g] = blk
        colpos += g

    deg_shard = np.ones((M, P, TILES), np.float32)
    deg_shard[core_of, p_of, t_of] = deg.astype(np.float32)

    x_pad = np.zeros((TABLE_ROWS, C), np.float32)
    x_pad[:N] = x

    return dict(
        groups=groups, SCOLS=SCOLS, idx_all=idx_all, deg_arr=deg_arr,
        deg_shard=deg_shard, x_pad=x_pad, core_of=core_of, pos_of=pos_of,
    )


def _build_program(groups, SCOLS, bias_zero):
    nc = bacc.Bacc("TRN2", target_bir_lowering=False, debug=False, num_devices=M)
    f32, f16, i32 = mybir.dt.float32, mybir.dt.float16, mybir.dt.int32

    x_d = nc.dram_tensor("x_pad", [TABLE_ROWS, C], f32, kind="ExternalInput")
    dega_d = nc.dram_tensor("deg_arr", [P, TABLE_STRIPS], f32, kind="ExternalInput")
    degs_d = nc.dram_tensor("deg_shard", [P, TILES], f32, kind="ExternalInput")
    idx_d = nc.dram_tensor("idx", [P, SCOLS], i32, kind="ExternalInput")
    w1_d = nc.dram_tensor("W1", [C, C], f32, kind="ExternalInput")
    w2_d = nc.dram_tensor("W2", [C, C], f32, kind="ExternalInput")
    b1_d = nc.dram_tensor("b1", [C], f32, kind="ExternalInput")
    b2_d = nc.dram_tensor("b2", [C], f32, kind="ExternalInput")
    table_d = nc.dram_tensor("table", [TABLE_ROWS, C], f16, kind="Internal")
    out_d = nc.dram_tensor("out", [P, TILES * C], f32, kind="ExternalOutput")

    with tile.TileContext(nc) as tc, ExitStack() as ctx:
        singles = ctx.enter_context(tc.tile_pool(name="singles", bufs=1))
        xpool = ctx.enter_context(tc.tile_pool(name="xin", bufs=3))
        tpool = ctx.enter_context(tc.tile_pool(name="tout", bufs=3))
        gpool = ctx.enter_context(tc.tile_pool(name="gather", bufs=2))
        lpool = ctx.enter_context(tc.tile_pool(name="lvl", bufs=2))
        apool = ctx.enter_context(tc.tile_pool(name="agg", bufs=3))
        tspool = ctx.enter_context(tc.tile_pool(name="trsb", bufs=3))
        pst = ctx.enter_context(tc.tile_pool(name="pst", bufs=3, space="PSUM"))
        psm = ctx.enter_context(tc.tile_pool(name="psm", bufs=3, space="PSUM"))

        # ---- singles ----
        w1_sb = singles.tile([C, C], f32)
        w2_sb = singles.tile([C, C], f32)
        nc.sync.dma_start(out=w1_sb[:], in_=w1_d.ap())
        nc.sync.dma_start(out=w2_sb[:], in_=w2_d.ap())

        # block-diagonal W4 [128, 128]: NT copies of Wsum on the diagonal
        w4a = singles.tile([P, P], f32)
        w4b = singles.tile([P, P], f32)
        nc.vector.memset(w4a[:], 0.0)
        nc.gpsimd.memset(w4b[:], 0.0)
        for t in range(NT):
            sl = slice(t * C, (t + 1) * C)
            nc.sync.dma_start(out=w4a[sl, sl], in_=w1_d.ap())
            nc.sync.dma_start(out=w4b[sl, sl], in_=w2_d.ap())
        nc.vector.tensor_add(out=w4a[:], in0=w4a[:], in1=w4b[:])

        if not bias_zero:
            b1_sb = singles.tile([P, C], f32)
            b2_sb = singles.tile([P, C], f32)
            nc.sync.dma_start(
                out=b1_sb[:], in_=bass.AP(tensor=b1_d, offset=0, ap=[[0, P], [1, C]])
            )
            nc.sync.dma_start(
                out=b2_sb[:], in_=bass.AP(tensor=b2_d, offset=0, ap=[[0, P], [1, C]])
            )
            bsum_sb = singles.tile([P, C], f32)
            nc.vector.tensor_add(out=bsum_sb[:], in0=b1_sb[:], in1=b2_sb[:])

        ident = singles.tile([P, P], f32)
        make_identity(nc, ident[:])

        idx_sb = singles.tile([P, SCOLS], i32)
        nc.sync.dma_start(out=idx_sb[:], in_=idx_d.ap())

        dinv_all = singles.tile([P, TABLE_STRIPS], f32)
        nc.sync.dma_start(out=dinv_all[:], in_=dega_d.ap())
        nc.scalar.sqrt(out=dinv_all[:], in_=dinv_all[:])
        nc.vector.reciprocal(out=dinv_all[:], in_=dinv_all[:])

        dinv_sh = singles.tile([P, TILES], f32)
        nc.sync.dma_start(out=dinv_sh[:], in_=degs_d.ap())
        nc.scalar.sqrt(out=dinv_sh[:], in_=dinv_sh[:])
        nc.vector.reciprocal(out=dinv_sh[:], in_=dinv_sh[:])

        out_all = singles.tile([P, TILES * C], f32)

        # ---- P0: table build, partition-contiguous strips ----
        colpos = 0
        si = 0
        for s0 in range(0, TABLE_STRIPS, G_STRIP):
            g = min(G_STRIP, TABLE_STRIPS - s0)
            # rows s0*128 + p*g + j  for j in [0, g)
            x_ap = bass.AP(
                tensor=x_d, offset=s0 * P * C,
                ap=[[g * C, P], [C, g], [1, C]],
            )
            t_ap = bass.AP(
                tensor=table_d, offset=s0 * P * C,
                ap=[[g * C, P], [C, g], [1, C]],
            )
            x_sb = xpool.tile([P, G_STRIP, C], f32, tag="x")
            nc.sync.dma_start(out=x_sb[:, :g, :], in_=x_ap)
            t_sb = tpool.tile([P, G_STRIP, C], f16, tag="t")
            eng = nc.gpsimd if si % 3 == 2 else nc.vector
            eng.tensor_tensor(
                out=t_sb[:, :g, :],
                in0=x_sb[:, :g, :],
                in1=dinv_all[:, colpos : colpos + g].to_broadcast([P, g, C]),
                op=mybir.AluOpType.mult,
            )
            nc.sync.dma_start(out=t_ap, in_=t_sb[:, :g, :])
            colpos += g
            si += 1

        # ---- P1 ----
        max_kg = max(Kg for _, _, Kg, _ in groups)
        for (g0, g1, Kg, c0) in groups:
            nt = g1 - g0
            # gather: one indirect DMA per slot column
            gbuf = gpool.tile([P, NT * max_kg * C], f16, tag="gbuf")
            ncols = nt * Kg
            for j in range(ncols):
                nc.gpsimd.indirect_dma_start(
                    out=gbuf[:, j * C : (j + 1) * C],
                    out_offset=None,
                    in_=table_d.ap(),
                    in_offset=bass.IndirectOffsetOnAxis(
                        ap=idx_sb[:, c0 + j : c0 + j + 1], axis=0
                    ),
                )
            # tree reduce: [p, nt, Kg, C] -> L1 -> L2 -> strided tail
            l1 = lpool.tile([P, NT * (max_kg // 2) * C], f16, tag="l1")
            v = gbuf[:, : nt * Kg * C].rearrange("p (t k c) -> p t k c", t=nt, c=C)
            nc.vector.tensor_tensor(
                out=l1[:, : nt * (Kg // 2) * C].rearrange(
                    "p (t k c) -> p t k c", t=nt, c=C
                ),
                in0=v[:, :, 0 : Kg : 2, :],
                in1=v[:, :, 1 : Kg : 2, :],
                op=mybir.AluOpType.add,
            )
            l2 = lpool.tile([P, NT * (max_kg // 4) * C], f16, tag="l2")
            v1 = l1[:, : nt * (Kg // 2) * C].rearrange(
                "p (t k c) -> p t k c", t=nt, c=C
            )
            nc.vector.tensor_tensor(
                out=l2[:, : nt * (Kg // 4) * C].rearrange(
                    "p (t k c) -> p t k c", t=nt, c=C
                ),
                in0=v1[:, :, 0 : Kg // 2 : 2, :],
                in1=v1[:, :, 1 : Kg // 2 : 2, :],
                op=mybir.AluOpType.add,
            )
            agg = apool.tile([P, NT * C], f32, tag="agg")
            v2 = l2[:, : nt * (Kg // 4) * C].rearrange(
                "p (t k c) -> p t c k", t=nt, c=C
            )
            nc.vector.tensor_reduce(
                out=agg[:, : nt * C].rearrange("p (t c) -> p t c", c=C),
                in_=v2, axis=mybir.AxisListType.X, op=mybir.AluOpType.add,
            )
            # dinv scale (per node row)
            nc.vector.tensor_tensor(
                out=agg[:, : nt * C].rearrange("p (t c) -> p t c", c=C),
                in0=agg[:, : nt * C].rearrange("p (t c) -> p t c", c=C),
                in1=dinv_sh[:, g0:g1].to_broadcast([P, nt, C]),
                op=mybir.AluOpType.mult,
            )
            # batched transpose + block-diag matmul
            trps = pst.tile([P, P], f32, tag="trps")
            nc.tensor.transpose(
                out=trps[: nt * C, :], in_=agg[:, : nt * C], identity=ident[:]
            )
            aggdT = tspool.tile([P, P], f32, tag="aggdT")
            nc.scalar.copy(out=aggdT[: nt * C, :], in_=trps[: nt * C, :])
            mm = psm.tile([P, P], f32, tag="mm")
            nc.tensor.matmul(
                out=mm[:, : nt * C],
                lhsT=aggdT[: nt * C, :],
                rhs=w4a[: nt * C, : nt * C],
                start=True, stop=True,
            )
            osl = out_all[:, g0 * C : g1 * C]
            if bias_zero:
                nc.scalar.activation(
                    out=osl, in_=mm[:, : nt * C],
                    func=mybir.ActivationFunctionType.Relu,
                )
            else:
                bb = bsum_sb[:].to_broadcast([P, nt, C]).rearrange("p t c -> p (t c)")
                nc.vector.tensor_tensor(
                    out=osl, in0=mm[:, : nt * C], in1=bb, op=mybir.AluOpType.add
                )
                nc.scalar.activation(
                    out=osl, in_=osl, func=mybir.ActivationFunctionType.Relu
                )

        nc.sync.dma_start(out=out_d.ap(), in_=out_all[:])

    nc.compile()
    return nc


_CACHE = {}


def _get_program(groups, SCOLS, bias_zero):
    key = (tuple(groups), SCOLS, bias_zero)
    if key not in _CACHE:
        _CACHE[key] = _build_program(groups, SCOLS, bias_zero)
    return _CACHE[key]


def run(x, edge_index, W1, b1, W2, b2, trace=False):
    prep = _host_prep(x, edge_index)
    bias_zero = not (np.any(np.asarray(b1)) or np.any(np.asarray(b2)))
    nc = _get_program(prep["groups"], prep["SCOLS"], bias_zero)

    W1 = np.ascontiguousarray(np.asarray(W1, np.float32))
    W2 = np.ascontiguousarray(np.asarray(W2, np.float32))
    b1 = np.ascontiguousarray(np.asarray(b1, np.float32))
    b2 = np.ascontiguousarray(np.asarray(b2, np.float32))

    in_maps = []
    for c in range(M):
        in_maps.append({
            "x_pad": prep["x_pad"],
            "deg_arr": prep["deg_arr"],
            "deg_shard": np.ascontiguousarray(prep["deg_shard"][c]),
            "idx": np.ascontiguousarray(prep["idx_all"][c]),
            "W1": W1, "W2": W2, "b1": b1, "b2": b2,
        })

    res = run_bass_kernel_spmd(nc, in_maps, core_ids=list(range(M)), trace=trace)

    # out[p, t*C + c] = node at (tile t, partition p)
    outs = np.stack(
        [res.results[c]["out"].reshape(P, TILES, C) for c in range(M)]
    )  # [M, P, TILES, C]
    t_of = prep["pos_of"] >> 7
    p_of = prep["pos_of"] & 127
    full = outs[prep["core_of"], p_of, t_of]
    return np.ascontiguousarray(full, dtype=np.float32), res


def kernel(x, edge_index, W1, b1, W2, b2):
    out, _ = run(x, edge_index, W1, b1, W2, b2, trace=False)
    return out


# revision 6
# speedup vs baseline: 1.0375x; 1.0375x over previous
"""Trainium2 Bass kernel for nn_DiffusionGraphConv_89936615178296.

out = relu(A_hat @ (x @ (W1+W2)) + b1 + b2), A_hat = D^-1/2 (A+I) D^-1/2.

Reformulation: out = relu(dinv * (Agg(dinv*x) @ W) + b), W = W1+W2,
dinv = rsqrt(1 + in_degree); the per-edge norm dinv[src]*dinv[dst] is
separable so per-edge work is a pure gather + segment-sum of x' = dinv*x.

8 cores, no collectives. Nodes sharded round-robin in degree-sorted order
(uniform slot schedule across cores, ~13% padding). Each core:
  P0: builds the fp16 gather table x' = dinv*x for the full graph in its
      local HBM (partition-contiguous strips).
  P1: per destination tile group, gathers source rows into a [128, nt, K,
      32] fp16 slot grid (one slot column per indirect DMA; each column
      supplies one dynamic row offset per partition), pairwise-tree
      reduces on the vector engine, applies dinv, multiplies by W on the
      tensor engine (batched transpose + block-diag matmul), relu, and
      accumulates the output shard in SBUF; one final DMA writes it out.

Host does integer index prep only (degree counts, shard permutation, CSR
slot layout); all f32 arithmetic runs on device.
"""

import numpy as np
from contextlib import ExitStack

import concourse.bass as bass
import concourse.bacc as bacc
import concourse.tile as tile
from concourse import mybir
from concourse.masks import make_identity
from concourse.bass_utils import run_bass_kernel_spmd

N, E, C = 100000, 1600000, 32
M = 8
P = 128
TILES = 98                  # 98*128 = 12544 rows per core >= 12500
ROWS_PER_CORE = TILES * P
TABLE_STRIPS = 783          # 783*128 = 100224 >= N+1 (zero row at index N)
TABLE_ROWS = TABLE_STRIPS * P
ZERO_ROW = N
G_STRIP = 64                # strip width (table rows per partition per strip)
NT = 4                      # tiles per group


def _host_prep(x, edge_index):
    src = np.asarray(edge_index[0], dtype=np.int64)
    dst = np.asarray(edge_index[1], dtype=np.int64)
    x = np.asarray(x, dtype=np.float32)

    deg = np.bincount(dst, minlength=N).astype(np.int64) + 1

    order = np.argsort(-deg, kind="stable")
    ar = np.arange(N)
    core_of = np.empty(N, np.int64)
    pos_of = np.empty(N, np.int64)
    core_of[order] = ar % M
    pos_of[order] = ar // M

    degs_sorted = deg[order]
    Kt = [int(degs_sorted[t * P * M]) if t * P * M < N else 1 for t in range(TILES)]
    # per-tile K rounded up to a multiple of 4 (clean 2-level pairwise tree)
    Kt = [4 * ((k + 3) // 4) for k in Kt]

    # groups of NT tiles for batched transpose/matmul; per-tile K kept tight
    groups = []  # (t0, t1, [K per tile], slot_col_offset)
    col = 0
    for g0 in range(0, TILES, NT):
        g1 = min(g0 + NT, TILES)
        groups.append((g0, g1, tuple(Kt[g0:g1]), col))
        col += sum(Kt[g0:g1])
    SCOLS = col  # total slot columns per core

    # CSR by destination
    eorder = np.argsort(dst, kind="stable")
    src_sorted = src[eorder].astype(np.int32)
    starts = np.concatenate([[0], np.cumsum(deg - 1)])

    t_of = pos_of >> 7
    p_of = pos_of & 127

    # column index of tile t's slot k
    colbase = np.empty(TILES, np.int64)
    for (g0, g1, Ks, c0) in groups:
        cc = c0
        for t in range(g0, g1):
            colbase[t] = cc
            cc += Ks[t - g0]

    idx_all = np.full((M, P, SCOLS), ZERO_ROW, np.int32)
    # self-loop at slot 0
    idx_all[core_of, p_of, colbase[t_of]] = np.arange(N, dtype=np.int32)
    # edges at slots 1..deg-1
    dstn = dst[eorder]
    rank = np.arange(E, dtype=np.int64) - starts[dstn]
    cols = colbase[t_of[dstn]] + 1 + rank
    idx_all[core_of[dstn], p_of[dstn], cols] = src_sorted

    deg_pad = np.ones(TABLE_ROWS, np.float32)
    deg_pad[:N] = deg
    # strip layout: strip s covers rows [s*128*G, ...), partition p gets rows
    # base + p*Gs + g (contiguous per partition)
    deg_arr = np.ones((P, TABLE_STRIPS), np.float32)
    colpos = 0
    for s0 in range(0, TABLE_STRIPS, G_STRIP):
        g = min(G_STRIP, TABLE_STRIPS - s0)
        blk = deg_pad[s0 * P : (s0 + g) * P].reshape(P, g)
        deg_arr[:, colpos : colpos + g] = blk
        colpos += g

    deg_shard = np.ones((M, P, TILES), np.float32)
    deg_shard[core_of, p_of, t_of] = deg.astype(np.float32)

    x_pad = np.zeros((TABLE_ROWS, C), np.float32)
    x_pad[:N] = x

    return dict(
        groups=groups, SCOLS=SCOLS, idx_all=idx_all, deg_arr=deg_arr,
        deg_shard=deg_shard, x_pad=x_pad, core_of=core_of, pos_of=pos_of,
    )


def _build_program(groups, SCOLS, bias_zero):
    nc = bacc.Bacc("TRN2", target_bir_lowering=False, debug=False, num_devices=M)
    f32, f16, i32 = mybir.dt.float32, mybir.dt.float16, mybir.dt.int32

    x_d = nc.dram_tensor("x_pad", [TABLE_ROWS, C], f32, kind="ExternalInput")
    dega_d = nc.dram_tensor("deg_arr", [P, TABLE_STRIPS], f32, kind="ExternalInput")
    degs_d = nc.dram_tensor("deg_shard", [P, TILES], f32, kind="ExternalInput")
    idx_d = nc.dram_tensor("idx", [P, SCOLS], i32, kind="ExternalInput")
    w1_d = nc.dram_tensor("W1", [C, C], f32, kind="ExternalInput")
    w2_d = nc.dram_tensor("W2", [C, C], f32, kind="ExternalInput")
    b1_d = nc.dram_tensor("b1", [C], f32, kind="ExternalInput")
    b2_d = nc.dram_tensor("b2", [C], f32, kind="ExternalInput")
    table_d = nc.dram_tensor("table", [TABLE_ROWS, C], f16, kind="Internal")
    out_d = nc.dram_tensor("out", [P, TILES * C], f32, kind="ExternalOutput")

    with tile.TileContext(nc) as tc, ExitStack() as ctx:
        singles = ctx.enter_context(tc.tile_pool(name="singles", bufs=1))
        xpool = ctx.enter_context(tc.tile_pool(name="xin", bufs=3))
        tpool = ctx.enter_context(tc.tile_pool(name="tout", bufs=3))
        gpool = ctx.enter_context(tc.tile_pool(name="gather", bufs=2))
        lpool = ctx.enter_context(tc.tile_pool(name="lvl", bufs=2))
        apool = ctx.enter_context(tc.tile_pool(name="agg", bufs=3))
        tspool = ctx.enter_context(tc.tile_pool(name="trsb", bufs=3))
        pst = ctx.enter_context(tc.tile_pool(name="pst", bufs=3, space="PSUM"))
        psm = ctx.enter_context(tc.tile_pool(name="psm", bufs=3, space="PSUM"))

        # ---- singles ----
        w1_sb = singles.tile([C, C], f32)
        w2_sb = singles.tile([C, C], f32)
        nc.sync.dma_start(out=w1_sb[:], in_=w1_d.ap())
        nc.sync.dma_start(out=w2_sb[:], in_=w2_d.ap())

        # block-diagonal W4 [128, 128]: NT copies of Wsum on the diagonal
        w4a = singles.tile([P, P], f32)
        w4b = singles.tile([P, P], f32)
        nc.vector.memset(w4a[:], 0.0)
        nc.gpsimd.memset(w4b[:], 0.0)
        for t in range(NT):
            sl = slice(t * C, (t + 1) * C)
            nc.sync.dma_start(out=w4a[sl, sl], in_=w1_d.ap())
            nc.sync.dma_start(out=w4b[sl, sl], in_=w2_d.ap())
        nc.vector.tensor_add(out=w4a[:], in0=w4a[:], in1=w4b[:])

        if not bias_zero:
            b1_sb = singles.tile([P, C], f32)
            b2_sb = singles.tile([P, C], f32)
            nc.sync.dma_start(
                out=b1_sb[:], in_=bass.AP(tensor=b1_d, offset=0, ap=[[0, P], [1, C]])
            )
            nc.sync.dma_start(
                out=b2_sb[:], in_=bass.AP(tensor=b2_d, offset=0, ap=[[0, P], [1, C]])
            )
            bsum_sb = singles.tile([P, C], f32)
            nc.vector.tensor_add(out=bsum_sb[:], in0=b1_sb[:], in1=b2_sb[:])

        ident = singles.tile([P, P], f32)
        make_identity(nc, ident[:])

        idx_sb = singles.tile([P, SCOLS], i32)
        nc.sync.dma_start(out=idx_sb[:], in_=idx_d.ap())

        dinv_all = singles.tile([P, TABLE_STRIPS], f32)
        nc.sync.dma_start(out=dinv_all[:], in_=dega_d.ap())
        nc.scalar.sqrt(out=dinv_all[:], in_=dinv_all[:])
        nc.vector.reciprocal(out=dinv_all[:], in_=dinv_all[:])

        dinv_sh = singles.tile([P, TILES], f32)
        nc.sync.dma_start(out=dinv_sh[:], in_=degs_d.ap())
        nc.scalar.sqrt(out=dinv_sh[:], in_=dinv_sh[:])
        nc.vector.reciprocal(out=dinv_sh[:], in_=dinv_sh[:])

        out_all = singles.tile([P, TILES * C], f32)

        # ---- P0: table build, partition-contiguous strips ----
        colpos = 0
        si = 0
        for s0 in range(0, TABLE_STRIPS, G_STRIP):
            g = min(G_STRIP, TABLE_STRIPS - s0)
            # rows s0*128 + p*g + j  for j in [0, g)
            x_ap = bass.AP(
                tensor=x_d, offset=s0 * P * C,
                ap=[[g * C, P], [C, g], [1, C]],
            )
            t_ap = bass.AP(
                tensor=table_d, offset=s0 * P * C,
                ap=[[g * C, P], [C, g], [1, C]],
            )
            x_sb = xpool.tile([P, G_STRIP, C], f32, tag="x")
            nc.sync.dma_start(out=x_sb[:, :g, :], in_=x_ap)
            t_sb = tpool.tile([P, G_STRIP, C], f16, tag="t")
            eng = nc.gpsimd if si % 3 == 2 else nc.vector
            eng.tensor_tensor(
                out=t_sb[:, :g, :],
                in0=x_sb[:, :g, :],
                in1=dinv_all[:, colpos : colpos + g].to_broadcast([P, g, C]),
                op=mybir.AluOpType.mult,
            )
            nc.sync.dma_start(out=t_ap, in_=t_sb[:, :g, :])
            colpos += g
            si += 1

        # ---- P1 ----
        max_sg = max(sum(Ks) for _, _, Ks, _ in groups)
        for (g0, g1, Ks, c0) in groups:
            nt = g1 - g0
            sg = sum(Ks)
            # gather: one indirect DMA per slot column
            gbuf = gpool.tile([P, max_sg * C], f16, tag="gbuf")
            for j in range(sg):
                nc.gpsimd.indirect_dma_start(
                    out=gbuf[:, j * C : (j + 1) * C],
                    out_offset=None,
                    in_=table_d.ap(),
                    in_offset=bass.IndirectOffsetOnAxis(
                        ap=idx_sb[:, c0 + j : c0 + j + 1], axis=0
                    ),
                )
            # per-tile pairwise tree: L1, L2 (fp16), then strided f32 tail
            l1 = lpool.tile([P, (max_sg // 2) * C], f16, tag="l1")
            l2 = lpool.tile([P, (max_sg // 4) * C], f16, tag="l2")
            agg = apool.tile([P, NT * C], f32, tag="agg")
            off = 0
            for ti in range(nt):
                K = Ks[ti]
                v = gbuf[:, off * C : (off + K) * C].rearrange(
                    "p (k c) -> p k c", c=C
                )
                o1 = l1[:, (off // 2) * C : ((off + K) // 2) * C].rearrange(
                    "p (k c) -> p k c", c=C
                )
                nc.vector.tensor_tensor(
                    out=o1, in0=v[:, 0:K:2, :], in1=v[:, 1:K:2, :],
                    op=mybir.AluOpType.add,
                )
                o2 = l2[:, (off // 4) * C : ((off + K) // 4) * C].rearrange(
                    "p (k c) -> p k c", c=C
                )
                v1 = l1[:, (off // 2) * C : ((off + K) // 2) * C].rearrange(
                    "p (k c) -> p k c", c=C
                )
                nc.vector.tensor_tensor(
                    out=o2, in0=v1[:, 0 : K // 2 : 2, :], in1=v1[:, 1 : K // 2 : 2, :],
                    op=mybir.AluOpType.add,
                )
                v2 = l2[:, (off // 4) * C : ((off + K) // 4) * C].rearrange(
                    "p (k c) -> p c k", c=C
                )
                nc.vector.tensor_reduce(
                    out=agg[:, ti * C : (ti + 1) * C],
                    in_=v2, axis=mybir.AxisListType.X, op=mybir.AluOpType.add,
                )
                off += K
            # dinv scale (per node row)
            nc.vector.tensor_tensor(
                out=agg[:, : nt * C].rearrange("p (t c) -> p t c", c=C),
                in0=agg[:, : nt * C].rearrange("p (t c) -> p t c", c=C),
                in1=dinv_sh[:, g0:g1].to_broadcast([P, nt, C]),
                op=mybir.AluOpType.mult,
            )
            # batched transpose + block-diag matmul
            trps = pst.tile([P, P], f32, tag="trps")
            nc.tensor.transpose(
                out=trps[: nt * C, :], in_=agg[:, : nt * C], identity=ident[:]
            )
            aggdT = tspool.tile([P, P], f32, tag="aggdT")
            nc.scalar.copy(out=aggdT[: nt * C, :], in_=trps[: nt * C, :])
            mm = psm.tile([P, P], f32, tag="mm")
            nc.tensor.matmul(
                out=mm[:, : nt * C],
                lhsT=aggdT[: nt * C, :],
                rhs=w4a[: nt * C, : nt * C],
                start=True, stop=True,
            )
            osl = out_all[:, g0 * C : g1 * C]
            if bias_zero:
                nc.scalar.activation(
                    out=osl, in_=mm[:, : nt * C],
                    func=mybir.ActivationFunctionType.Relu,
                )
            else:
                for ti in range(nt):
                    nc.vector.tensor_add(
                        out=osl[:, ti * C : (ti + 1) * C],
                        in0=mm[:, ti * C : (ti + 1) * C],
                        in1=bsum_sb[:],
                    )
                nc.scalar.activation(
                    out=osl, in_=osl, func=mybir.ActivationFunctionType.Relu
                )

        nc.sync.dma_start(out=out_d.ap(), in_=out_all[:])

    nc.compile()
    return nc


_CACHE = {}


def _get_program(groups, SCOLS, bias_zero):
    key = (tuple(groups), SCOLS, bias_zero)
    if key not in _CACHE:
        _CACHE[key] = _build_program(groups, SCOLS, bias_zero)
    return _CACHE[key]


def run(x, edge_index, W1, b1, W2, b2, trace=False):
    prep = _host_prep(x, edge_index)
    bias_zero = not (np.any(np.asarray(b1)) or np.any(np.asarray(b2)))
    nc = _get_program(prep["groups"], prep["SCOLS"], bias_zero)

    W1 = np.ascontiguousarray(np.asarray(W1, np.float32))
    W2 = np.ascontiguousarray(np.asarray(W2, np.float32))
    b1 = np.ascontiguousarray(np.asarray(b1, np.float32))
    b2 = np.ascontiguousarray(np.asarray(b2, np.float32))

    in_maps = []
    for c in range(M):
        in_maps.append({
            "x_pad": prep["x_pad"],
            "deg_arr": prep["deg_arr"],
            "deg_shard": np.ascontiguousarray(prep["deg_shard"][c]),
            "idx": np.ascontiguousarray(prep["idx_all"][c]),
            "W1": W1, "W2": W2, "b1": b1, "b2": b2,
        })

    res = run_bass_kernel_spmd(nc, in_maps, core_ids=list(range(M)), trace=trace)

    # out[p, t*C + c] = node at (tile t, partition p)
    outs = np.stack(
        [res.results[c]["out"].reshape(P, TILES, C) for c in range(M)]
    )  # [M, P, TILES, C]
    t_of = prep["pos_of"] >> 7
    p_of = prep["pos_of"] & 127
    full = outs[prep["core_of"], p_of, t_of]
    return np.ascontiguousarray(full, dtype=np.float32), res


def kernel(x, edge_index, W1, b1, W2, b2):
    out, _ = run(x, edge_index, W1, b1, W2, b2, trace=False)
    return out


# revision 8
# speedup vs baseline: 1.1014x; 1.0616x over previous
"""Trainium2 Bass kernel for nn_DiffusionGraphConv_89936615178296.

out = relu(A_hat @ (x @ (W1+W2)) + b1 + b2), A_hat = D^-1/2 (A+I) D^-1/2.

Reformulation: out = relu(dinv * (Agg(dinv*x) @ W) + b), W = W1+W2,
dinv = rsqrt(1 + in_degree); the per-edge norm dinv[src]*dinv[dst] is
separable so per-edge work is a pure gather + segment-sum of x' = dinv*x.

8 cores, no collectives. Nodes sharded round-robin in degree-sorted order
(uniform slot schedule across cores, ~13% padding). Each core:
  P0: builds the fp16 gather table x' = dinv*x for the full graph in its
      local HBM (partition-contiguous strips).
  P1: per destination tile group, gathers source rows into a [128, nt, K,
      32] fp16 slot grid (one slot column per indirect DMA; each column
      supplies one dynamic row offset per partition), pairwise-tree
      reduces on the vector engine, applies dinv, multiplies by W on the
      tensor engine (batched transpose + block-diag matmul), relu, and
      accumulates the output shard in SBUF; one final DMA writes it out.

Host does integer index prep only (degree counts, shard permutation, CSR
slot layout); all f32 arithmetic runs on device.
"""

import numpy as np
from contextlib import ExitStack

import concourse.bass as bass
import concourse.bacc as bacc
import concourse.tile as tile
from concourse import mybir
from concourse.masks import make_identity
from concourse.bass_utils import run_bass_kernel_spmd

N, E, C = 100000, 1600000, 32
M = 8
P = 128
TILES = 98                  # 98*128 = 12544 rows per core >= 12500
ROWS_PER_CORE = TILES * P
TABLE_STRIPS = 783          # 783*128 = 100224 >= N+1 (zero row at index N)
TABLE_ROWS = TABLE_STRIPS * P
ZERO_ROW = N
G_STRIP = 64                # strip width (table rows per partition per strip)
NT = 4                      # tiles per group


def _host_prep(x, edge_index):
    src = np.asarray(edge_index[0], dtype=np.int64)
    dst = np.asarray(edge_index[1], dtype=np.int64)
    x = np.asarray(x, dtype=np.float32)

    deg = np.bincount(dst, minlength=N).astype(np.int64) + 1

    order = np.argsort(-deg, kind="stable")
    ar = np.arange(N)
    core_of = np.empty(N, np.int64)
    pos_of = np.empty(N, np.int64)
    core_of[order] = ar % M
    pos_of[order] = ar // M

    degs_sorted = deg[order]
    Kt = [int(degs_sorted[t * P * M]) if t * P * M < N else 1 for t in range(TILES)]
    # per-tile K rounded up to even (clean first pairwise-tree level)
    Kt = [2 * ((k + 1) // 2) for k in Kt]

    # groups of NT tiles for batched transpose/matmul; per-tile K kept tight
    groups = []  # (t0, t1, [K per tile], slot_col_offset)
    col = 0
    for g0 in range(0, TILES, NT):
        g1 = min(g0 + NT, TILES)
        groups.append((g0, g1, tuple(Kt[g0:g1]), col))
        col += sum(Kt[g0:g1])
    SCOLS = col  # total slot columns per core

    # CSR by destination
    eorder = np.argsort(dst, kind="stable")
    src_sorted = src[eorder].astype(np.int32)
    starts = np.concatenate([[0], np.cumsum(deg - 1)])

    t_of = pos_of >> 7
    p_of = pos_of & 127

    # column index of tile t's slot k
    colbase = np.empty(TILES, np.int64)
    for (g0, g1, Ks, c0) in groups:
        cc = c0
        for t in range(g0, g1):
            colbase[t] = cc
            cc += Ks[t - g0]

    idx_all = np.full((M, P, SCOLS), ZERO_ROW, np.int32)
    # self-loop at slot 0
    idx_all[core_of, p_of, colbase[t_of]] = np.arange(N, dtype=np.int32)
    # edges at slots 1..deg-1
    dstn = dst[eorder]
    rank = np.arange(E, dtype=np.int64) - starts[dstn]
    cols = colbase[t_of[dstn]] + 1 + rank
    idx_all[core_of[dstn], p_of[dstn], cols] = src_sorted

    deg_pad = np.ones(TABLE_ROWS, np.float32)
    deg_pad[:N] = deg
    # strip layout: strip s covers rows [s*128*G, ...), partition p gets rows
    # base + p*Gs + g (contiguous per partition)
    deg_arr = np.ones((P, TABLE_STRIPS), np.float32)
    colpos = 0
    for s0 in range(0, TABLE_STRIPS, G_STRIP):
        g = min(G_STRIP, TABLE_STRIPS - s0)
        blk = deg_pad[s0 * P : (s0 + g) * P].reshape(P, g)
        deg_arr[:, colpos : colpos + g] = blk
        colpos += g

    deg_shard = np.ones((M, P, TILES), np.float32)
    deg_shard[core_of, p_of, t_of] = deg.astype(np.float32)

    x_pad = np.zeros((TABLE_ROWS, C), np.float32)
    x_pad[:N] = x

    return dict(
        groups=groups, SCOLS=SCOLS, idx_all=idx_all, deg_arr=deg_arr,
        deg_shard=deg_shard, x_pad=x_pad, core_of=core_of, pos_of=pos_of,
    )


def _build_program(groups, SCOLS, bias_zero):
    nc = bacc.Bacc("TRN2", target_bir_lowering=False, debug=False, num_devices=M)
    f32, f16, i32 = mybir.dt.float32, mybir.dt.float16, mybir.dt.int32

    x_d = nc.dram_tensor("x_pad", [TABLE_ROWS, C], f32, kind="ExternalInput")
    dega_d = nc.dram_tensor("deg_arr", [P, TABLE_STRIPS], f32, kind="ExternalInput")
    degs_d = nc.dram_tensor("deg_shard", [P, TILES], f32, kind="ExternalInput")
    idx_d = nc.dram_tensor("idx", [P, SCOLS], i32, kind="ExternalInput")
    w1_d = nc.dram_tensor("W1", [C, C], f32, kind="ExternalInput")
    w2_d = nc.dram_tensor("W2", [C, C], f32, kind="ExternalInput")
    b1_d = nc.dram_tensor("b1", [C], f32, kind="ExternalInput")
    b2_d = nc.dram_tensor("b2", [C], f32, kind="ExternalInput")
    table_d = nc.dram_tensor("table", [TABLE_ROWS, C], f16, kind="Internal")
    out_d = nc.dram_tensor("out", [P, TILES * C], f32, kind="ExternalOutput")

    with tile.TileContext(nc) as tc, ExitStack() as ctx:
        singles = ctx.enter_context(tc.tile_pool(name="singles", bufs=1))
        xpool = ctx.enter_context(tc.tile_pool(name="xin", bufs=3))
        tpool = ctx.enter_context(tc.tile_pool(name="tout", bufs=3))
        gpool = ctx.enter_context(tc.tile_pool(name="gather", bufs=2))
        lpool = ctx.enter_context(tc.tile_pool(name="lvl", bufs=2))
        apool = ctx.enter_context(tc.tile_pool(name="agg", bufs=3))
        tspool = ctx.enter_context(tc.tile_pool(name="trsb", bufs=3))
        pst = ctx.enter_context(tc.tile_pool(name="pst", bufs=3, space="PSUM"))
        psm = ctx.enter_context(tc.tile_pool(name="psm", bufs=3, space="PSUM"))

        # ---- singles ----
        w1_sb = singles.tile([C, C], f32)
        w2_sb = singles.tile([C, C], f32)
        nc.sync.dma_start(out=w1_sb[:], in_=w1_d.ap())
        nc.sync.dma_start(out=w2_sb[:], in_=w2_d.ap())

        # block-diagonal W4 [128, 128]: NT copies of Wsum on the diagonal
        w4a = singles.tile([P, P], f32)
        w4b = singles.tile([P, P], f32)
        nc.vector.memset(w4a[:], 0.0)
        nc.gpsimd.memset(w4b[:], 0.0)
        for t in range(NT):
            sl = slice(t * C, (t + 1) * C)
            nc.sync.dma_start(out=w4a[sl, sl], in_=w1_d.ap())
            nc.sync.dma_start(out=w4b[sl, sl], in_=w2_d.ap())
        nc.vector.tensor_add(out=w4a[:], in0=w4a[:], in1=w4b[:])

        if not bias_zero:
            b1_sb = singles.tile([P, C], f32)
            b2_sb = singles.tile([P, C], f32)
            nc.sync.dma_start(
                out=b1_sb[:], in_=bass.AP(tensor=b1_d, offset=0, ap=[[0, P], [1, C]])
            )
            nc.sync.dma_start(
                out=b2_sb[:], in_=bass.AP(tensor=b2_d, offset=0, ap=[[0, P], [1, C]])
            )
            bsum_sb = singles.tile([P, C], f32)
            nc.vector.tensor_add(out=bsum_sb[:], in0=b1_sb[:], in1=b2_sb[:])

        ident = singles.tile([P, P], f32)
        make_identity(nc, ident[:])

        idx_sb = singles.tile([P, SCOLS], i32)
        nc.sync.dma_start(out=idx_sb[:], in_=idx_d.ap())

        dinv_all = singles.tile([P, TABLE_STRIPS], f32)
        nc.sync.dma_start(out=dinv_all[:], in_=dega_d.ap())
        nc.scalar.sqrt(out=dinv_all[:], in_=dinv_all[:])
        nc.vector.reciprocal(out=dinv_all[:], in_=dinv_all[:])

        dinv_sh = singles.tile([P, TILES], f32)
        nc.sync.dma_start(out=dinv_sh[:], in_=degs_d.ap())
        nc.scalar.sqrt(out=dinv_sh[:], in_=dinv_sh[:])
        nc.vector.reciprocal(out=dinv_sh[:], in_=dinv_sh[:])

        out_all = singles.tile([P, TILES * C], f32)

        # ---- P0: table build, partition-contiguous strips ----
        colpos = 0
        si = 0
        for s0 in range(0, TABLE_STRIPS, G_STRIP):
            g = min(G_STRIP, TABLE_STRIPS - s0)
            # rows s0*128 + p*g + j  for j in [0, g)
            x_ap = bass.AP(
                tensor=x_d, offset=s0 * P * C,
                ap=[[g * C, P], [C, g], [1, C]],
            )
            t_ap = bass.AP(
                tensor=table_d, offset=s0 * P * C,
                ap=[[g * C, P], [C, g], [1, C]],
            )
            x_sb = xpool.tile([P, G_STRIP, C], f32, tag="x")
            nc.sync.dma_start(out=x_sb[:, :g, :], in_=x_ap)
            t_sb = tpool.tile([P, G_STRIP, C], f16, tag="t")
            eng = nc.gpsimd if si % 3 == 2 else nc.vector
            eng.tensor_tensor(
                out=t_sb[:, :g, :],
                in0=x_sb[:, :g, :],
                in1=dinv_all[:, colpos : colpos + g].to_broadcast([P, g, C]),
                op=mybir.AluOpType.mult,
            )
            nc.sync.dma_start(out=t_ap, in_=t_sb[:, :g, :])
            colpos += g
            si += 1

        # ---- P1 ----
        max_sg = max(sum(Ks) for _, _, Ks, _ in groups)
        for (g0, g1, Ks, c0) in groups:
            nt = g1 - g0
            sg = sum(Ks)
            # gather: one indirect DMA per slot column
            gbuf = gpool.tile([P, max_sg * C], f16, tag="gbuf")
            for j in range(sg):
                nc.gpsimd.indirect_dma_start(
                    out=gbuf[:, j * C : (j + 1) * C],
                    out_offset=None,
                    in_=table_d.ap(),
                    in_offset=bass.IndirectOffsetOnAxis(
                        ap=idx_sb[:, c0 + j : c0 + j + 1], axis=0
                    ),
                )
            # per-tile pairwise tree: L1, L2 (fp16), then strided f32 tail
            l1 = lpool.tile([P, (max_sg // 2) * C], f16, tag="l1")
            l2 = lpool.tile([P, (max_sg // 4) * C], f16, tag="l2")
            agg = apool.tile([P, NT * C], f32, tag="agg")
            off = 0   # slot-column offset within gbuf
            hoff = 0  # half-column offset within l1
            qoff = 0  # quarter-column offset within l2
            for ti in range(nt):
                K = Ks[ti]
                K2 = K // 2
                v = gbuf[:, off * C : (off + K) * C].rearrange(
                    "p (k c) -> p k c", c=C
                )
                o1 = l1[:, hoff * C : (hoff + K2) * C].rearrange(
                    "p (k c) -> p k c", c=C
                )
                nc.vector.tensor_tensor(
                    out=o1, in0=v[:, 0:K:2, :], in1=v[:, 1:K:2, :],
                    op=mybir.AluOpType.add,
                )
                v1 = l1[:, hoff * C : (hoff + K2) * C]
                if K2 % 2 == 0 and K2 >= 4:
                    K4 = K2 // 2
                    o2 = l2[:, qoff * C : (qoff + K4) * C].rearrange(
                        "p (k c) -> p k c", c=C
                    )
                    v1r = v1.rearrange("p (k c) -> p k c", c=C)
                    nc.vector.tensor_tensor(
                        out=o2, in0=v1r[:, 0:K2:2, :], in1=v1r[:, 1:K2:2, :],
                        op=mybir.AluOpType.add,
                    )
                    tail = l2[:, qoff * C : (qoff + K4) * C].rearrange(
                        "p (k c) -> p c k", c=C
                    )
                    qoff += K4
                else:
                    tail = v1.rearrange("p (k c) -> p c k", c=C)
                nc.vector.tensor_reduce(
                    out=agg[:, ti * C : (ti + 1) * C],
                    in_=tail, axis=mybir.AxisListType.X, op=mybir.AluOpType.add,
                )
                off += K
                hoff += K2
            # dinv scale (per node row)
            nc.vector.tensor_tensor(
                out=agg[:, : nt * C].rearrange("p (t c) -> p t c", c=C),
                in0=agg[:, : nt * C].rearrange("p (t c) -> p t c", c=C),
                in1=dinv_sh[:, g0:g1].to_broadcast([P, nt, C]),
                op=mybir.AluOpType.mult,
            )
            # batched transpose + block-diag matmul
            trps = pst.tile([P, P], f32, tag="trps")
            nc.tensor.transpose(
                out=trps[: nt * C, :], in_=agg[:, : nt * C], identity=ident[:]
            )
            aggdT = tspool.tile([P, P], f32, tag="aggdT")
            nc.scalar.copy(out=aggdT[: nt * C, :], in_=trps[: nt * C, :])
            mm = psm.tile([P, P], f32, tag="mm")
            nc.tensor.matmul(
                out=mm[:, : nt * C],
                lhsT=aggdT[: nt * C, :],
                rhs=w4a[: nt * C, : nt * C],
                start=True, stop=True,
            )
            osl = out_all[:, g0 * C : g1 * C]
            if bias_zero:
                nc.scalar.activation(
                    out=osl, in_=mm[:, : nt * C],
                    func=mybir.ActivationFunctionType.Relu,
                )
            else:
                for ti in range(nt):
                    nc.vector.tensor_add(
                        out=osl[:, ti * C : (ti + 1) * C],
                        in0=mm[:, ti * C : (ti + 1) * C],
                        in1=bsum_sb[:],
                    )
                nc.scalar.activation(
                    out=osl, in_=osl, func=mybir.ActivationFunctionType.Relu
                )

        nc.sync.dma_start(out=out_d.ap(), in_=out_all[:])

    nc.compile()
    return nc


_CACHE = {}


def _get_program(groups, SCOLS, bias_zero):
    key = (tuple(groups), SCOLS, bias_zero)
    if key not in _CACHE:
        _CACHE[key] = _build_program(groups, SCOLS, bias_zero)
    return _CACHE[key]


def run(x, edge_index, W1, b1, W2, b2, trace=False):
    prep = _host_prep(x, edge_index)
    bias_zero = not (np.any(np.asarray(b1)) or np.any(np.asarray(b2)))
    nc = _get_program(prep["groups"], prep["SCOLS"], bias_zero)

    W1 = np.ascontiguousarray(np.asarray(W1, np.float32))
    W2 = np.ascontiguousarray(np.asarray(W2, np.float32))
    b1 = np.ascontiguousarray(np.asarray(b1, np.float32))
    b2 = np.ascontiguousarray(np.asarray(b2, np.float32))

    in_maps = []
    for c in range(M):
        in_maps.append({
            "x_pad": prep["x_pad"],
            "deg_arr": prep["deg_arr"],
            "deg_shard": np.ascontiguousarray(prep["deg_shard"][c]),
            "idx": np.ascontiguousarray(prep["idx_all"][c]),
            "W1": W1, "W2": W2, "b1": b1, "b2": b2,
        })

    res = run_bass_kernel_spmd(nc, in_maps, core_ids=list(range(M)), trace=trace)

    # out[p, t*C + c] = node at (tile t, partition p)
    outs = np.stack(
        [res.results[c]["out"].reshape(P, TILES, C) for c in range(M)]
    )  # [M, P, TILES, C]
    t_of = prep["pos_of"] >> 7
    p_of = prep["pos_of"] & 127
    full = outs[prep["core_of"], p_of, t_of]
    return np.ascontiguousarray(full, dtype=np.float32), res


def kernel(x, edge_index, W1, b1, W2, b2):
    out, _ = run(x, edge_index, W1, b1, W2, b2, trace=False)
    return out


# revision 9
# speedup vs baseline: 1.1662x; 1.0588x over previous
"""Trainium2 Bass kernel for nn_DiffusionGraphConv_89936615178296.

out = relu(A_hat @ (x @ (W1+W2)) + b1 + b2), A_hat = D^-1/2 (A+I) D^-1/2.

Reformulation: out = relu(dinv * (Agg(dinv*x) @ W) + b), W = W1+W2,
dinv = rsqrt(1 + in_degree); the per-edge norm dinv[src]*dinv[dst] is
separable so per-edge work is a pure gather + segment-sum of x' = dinv*x.

8 cores, no collectives. Nodes sharded round-robin in degree-sorted order
(uniform slot schedule across cores, ~13% padding). Each core:
  P0: builds the fp16 gather table x' = dinv*x for the full graph in its
      local HBM (partition-contiguous strips).
  P1: per destination tile group, gathers source rows into a [128, nt, K,
      32] fp16 slot grid (one slot column per indirect DMA; each column
      supplies one dynamic row offset per partition), pairwise-tree
      reduces on the vector engine, applies dinv, multiplies by W on the
      tensor engine (batched transpose + block-diag matmul), relu, and
      accumulates the output shard in SBUF; one final DMA writes it out.

Host does integer index prep only (degree counts, shard permutation, CSR
slot layout); all f32 arithmetic runs on device.
"""

import numpy as np
from contextlib import ExitStack

import concourse.bass as bass
import concourse.bacc as bacc
import concourse.tile as tile
from concourse import mybir
from concourse.masks import make_identity
from concourse.bass_utils import run_bass_kernel_spmd

N, E, C = 100000, 1600000, 32
M = 8
P = 128
TILES = 98                  # 98*128 = 12544 rows per core >= 12500
ROWS_PER_CORE = TILES * P
TABLE_STRIPS = 783          # 783*128 = 100224 >= N+1 (zero row at index N)
TABLE_ROWS = TABLE_STRIPS * P
ZERO_ROW = N
G_STRIP = 64                # strip width (table rows per partition per strip)
NT = 4                      # tiles per group


def _host_prep(x, edge_index):
    src = np.asarray(edge_index[0], dtype=np.int64)
    dst = np.asarray(edge_index[1], dtype=np.int64)
    x = np.asarray(x, dtype=np.float32)

    deg = np.bincount(dst, minlength=N).astype(np.int64) + 1

    order = np.argsort(-deg, kind="stable")
    ar = np.arange(N)
    core_of = np.empty(N, np.int64)
    pos_of = np.empty(N, np.int64)
    core_of[order] = ar % M
    pos_of[order] = ar // M

    degs_sorted = deg[order]
    # edges only (self-loop handled via x_shard), even-rounded, >= 2
    Kt = [int(degs_sorted[t * P * M]) - 1 if t * P * M < N else 1 for t in range(TILES)]
    Kt = [max(2, 2 * ((k + 1) // 2)) for k in Kt]

    # groups of NT tiles for batched transpose/matmul; per-tile K kept tight
    groups = []  # (t0, t1, [K per tile], slot_col_offset)
    col = 0
    for g0 in range(0, TILES, NT):
        g1 = min(g0 + NT, TILES)
        groups.append((g0, g1, tuple(Kt[g0:g1]), col))
        col += sum(Kt[g0:g1])
    SCOLS = col  # total slot columns per core

    # CSR by destination
    eorder = np.argsort(dst, kind="stable")
    src_sorted = src[eorder].astype(np.int32)
    starts = np.concatenate([[0], np.cumsum(deg - 1)])

    t_of = pos_of >> 7
    p_of = pos_of & 127

    # column index of tile t's slot k
    colbase = np.empty(TILES, np.int64)
    for (g0, g1, Ks, c0) in groups:
        cc = c0
        for t in range(g0, g1):
            colbase[t] = cc
            cc += Ks[t - g0]

    idx_all = np.full((M, P, SCOLS), ZERO_ROW, np.int32)
    # edges at slots 0..deg-2 (self-loop contributed via x_shard)
    dstn = dst[eorder]
    rank = np.arange(E, dtype=np.int64) - starts[dstn]
    cols = colbase[t_of[dstn]] + rank
    idx_all[core_of[dstn], p_of[dstn], cols] = src_sorted

    # per-core own-shard features, [P, TILES, C], zero on pad rows
    x_shard = np.zeros((M, P, TILES, C), np.float32)
    x_shard[core_of, p_of, t_of] = x

    deg_pad = np.ones(TABLE_ROWS, np.float32)
    deg_pad[:N] = deg
    # strip layout: strip s covers rows [s*128*G, ...), partition p gets rows
    # base + p*Gs + g (contiguous per partition)
    deg_arr = np.ones((P, TABLE_STRIPS), np.float32)
    colpos = 0
    for s0 in range(0, TABLE_STRIPS, G_STRIP):
        g = min(G_STRIP, TABLE_STRIPS - s0)
        blk = deg_pad[s0 * P : (s0 + g) * P].reshape(P, g)
        deg_arr[:, colpos : colpos + g] = blk
        colpos += g

    deg_shard = np.ones((M, P, TILES), np.float32)
    deg_shard[core_of, p_of, t_of] = deg.astype(np.float32)

    x_pad = np.zeros((TABLE_ROWS, C), np.float32)
    x_pad[:N] = x

    return dict(
        groups=groups, SCOLS=SCOLS, idx_all=idx_all, deg_arr=deg_arr,
        deg_shard=deg_shard, x_pad=x_pad, core_of=core_of, pos_of=pos_of,
        x_shard=x_shard,
    )


def _build_program(groups, SCOLS, bias_zero):
    nc = bacc.Bacc("TRN2", target_bir_lowering=False, debug=False, num_devices=M)
    f32, f16, i32 = mybir.dt.float32, mybir.dt.float16, mybir.dt.int32

    x_d = nc.dram_tensor("x_pad", [TABLE_ROWS, C], f32, kind="ExternalInput")
    dega_d = nc.dram_tensor("deg_arr", [P, TABLE_STRIPS], f32, kind="ExternalInput")
    degs_d = nc.dram_tensor("deg_shard", [P, TILES], f32, kind="ExternalInput")
    idx_d = nc.dram_tensor("idx", [P, SCOLS], i32, kind="ExternalInput")
    xs_d = nc.dram_tensor("x_shard", [P, TILES * C], f32, kind="ExternalInput")
    w1_d = nc.dram_tensor("W1", [C, C], f32, kind="ExternalInput")
    w2_d = nc.dram_tensor("W2", [C, C], f32, kind="ExternalInput")
    b1_d = nc.dram_tensor("b1", [C], f32, kind="ExternalInput")
    b2_d = nc.dram_tensor("b2", [C], f32, kind="ExternalInput")
    table_d = nc.dram_tensor("table", [TABLE_ROWS, C], f16, kind="Internal")
    out_d = nc.dram_tensor("out", [P, TILES * C], f32, kind="ExternalOutput")

    with tile.TileContext(nc) as tc, ExitStack() as ctx:
        singles = ctx.enter_context(tc.tile_pool(name="singles", bufs=1))
        xpool = ctx.enter_context(tc.tile_pool(name="xin", bufs=3))
        tpool = ctx.enter_context(tc.tile_pool(name="tout", bufs=3))
        gpool = ctx.enter_context(tc.tile_pool(name="gather", bufs=3))
        lpool = ctx.enter_context(tc.tile_pool(name="lvl", bufs=2))
        apool = ctx.enter_context(tc.tile_pool(name="agg", bufs=3))
        tspool = ctx.enter_context(tc.tile_pool(name="trsb", bufs=3))
        pst = ctx.enter_context(tc.tile_pool(name="pst", bufs=3, space="PSUM"))
        psm = ctx.enter_context(tc.tile_pool(name="psm", bufs=3, space="PSUM"))

        # ---- singles ----
        w1_sb = singles.tile([C, C], f32)
        w2_sb = singles.tile([C, C], f32)
        nc.sync.dma_start(out=w1_sb[:], in_=w1_d.ap())
        nc.sync.dma_start(out=w2_sb[:], in_=w2_d.ap())

        # block-diagonal W4 [128, 128]: NT copies of Wsum on the diagonal
        w4a = singles.tile([P, P], f32)
        w4b = singles.tile([P, P], f32)
        nc.vector.memset(w4a[:], 0.0)
        nc.gpsimd.memset(w4b[:], 0.0)
        for t in range(NT):
            sl = slice(t * C, (t + 1) * C)
            nc.sync.dma_start(out=w4a[sl, sl], in_=w1_d.ap())
            nc.sync.dma_start(out=w4b[sl, sl], in_=w2_d.ap())
        nc.vector.tensor_add(out=w4a[:], in0=w4a[:], in1=w4b[:])

        if not bias_zero:
            b1_sb = singles.tile([P, C], f32)
            b2_sb = singles.tile([P, C], f32)
            nc.sync.dma_start(
                out=b1_sb[:], in_=bass.AP(tensor=b1_d, offset=0, ap=[[0, P], [1, C]])
            )
            nc.sync.dma_start(
                out=b2_sb[:], in_=bass.AP(tensor=b2_d, offset=0, ap=[[0, P], [1, C]])
            )
            bsum_sb = singles.tile([P, C], f32)
            nc.vector.tensor_add(out=bsum_sb[:], in0=b1_sb[:], in1=b2_sb[:])

        ident = singles.tile([P, P], f32)
        make_identity(nc, ident[:])

        idx_sb = singles.tile([P, SCOLS], i32)
        nc.sync.dma_start(out=idx_sb[:], in_=idx_d.ap())

        dinv_all = singles.tile([P, TABLE_STRIPS], f32)
        nc.sync.dma_start(out=dinv_all[:], in_=dega_d.ap())
        nc.scalar.sqrt(out=dinv_all[:], in_=dinv_all[:])
        nc.vector.reciprocal(out=dinv_all[:], in_=dinv_all[:])

        dinv_sh = singles.tile([P, TILES], f32)
        nc.sync.dma_start(out=dinv_sh[:], in_=degs_d.ap())
        nc.scalar.sqrt(out=dinv_sh[:], in_=dinv_sh[:])
        nc.vector.reciprocal(out=dinv_sh[:], in_=dinv_sh[:])

        out_all = singles.tile([P, TILES * C], f32)

        # self-loop contribution x' = dinv * x for own shard rows
        xs_all = singles.tile([P, TILES * C], f32)
        nc.sync.dma_start(out=xs_all[:], in_=xs_d.ap())
        nc.vector.tensor_tensor(
            out=xs_all[:].rearrange("p (t c) -> p t c", c=C),
            in0=xs_all[:].rearrange("p (t c) -> p t c", c=C),
            in1=dinv_sh[:].to_broadcast([P, TILES, C]),
            op=mybir.AluOpType.mult,
        )

        # ---- P0: table build, partition-contiguous strips ----
        colpos = 0
        si = 0
        for s0 in range(0, TABLE_STRIPS, G_STRIP):
            g = min(G_STRIP, TABLE_STRIPS - s0)
            # rows s0*128 + p*g + j  for j in [0, g)
            x_ap = bass.AP(
                tensor=x_d, offset=s0 * P * C,
                ap=[[g * C, P], [C, g], [1, C]],
            )
            t_ap = bass.AP(
                tensor=table_d, offset=s0 * P * C,
                ap=[[g * C, P], [C, g], [1, C]],
            )
            x_sb = xpool.tile([P, G_STRIP, C], f32, tag="x")
            nc.sync.dma_start(out=x_sb[:, :g, :], in_=x_ap)
            t_sb = tpool.tile([P, G_STRIP, C], f16, tag="t")
            nc.vector.tensor_tensor(
                out=t_sb[:, :g, :],
                in0=x_sb[:, :g, :],
                in1=dinv_all[:, colpos : colpos + g].to_broadcast([P, g, C]),
                op=mybir.AluOpType.mult,
            )
            nc.sync.dma_start(out=t_ap, in_=t_sb[:, :g, :])
            colpos += g
            si += 1

        # ---- P1 ----
        max_sg = max(sum(Ks) for _, _, Ks, _ in groups)
        for (g0, g1, Ks, c0) in groups:
            nt = g1 - g0
            sg = sum(Ks)
            # gather: one indirect DMA per slot column
            gbuf = gpool.tile([P, max_sg * C], f16, tag="gbuf")
            for j in range(sg):
                nc.gpsimd.indirect_dma_start(
                    out=gbuf[:, j * C : (j + 1) * C],
                    out_offset=None,
                    in_=table_d.ap(),
                    in_offset=bass.IndirectOffsetOnAxis(
                        ap=idx_sb[:, c0 + j : c0 + j + 1], axis=0
                    ),
                )
            # per-tile pairwise tree: L1, L2 (fp16), then strided f32 tail
            l1 = lpool.tile([P, (max_sg // 2) * C], f16, tag="l1")
            l2 = lpool.tile([P, (max_sg // 4) * C], f16, tag="l2")
            agg = apool.tile([P, NT * C], f32, tag="agg")
            off = 0   # slot-column offset within gbuf
            hoff = 0  # half-column offset within l1
            qoff = 0  # quarter-column offset within l2
            for ti in range(nt):
                K = Ks[ti]
                K2 = K // 2
                v = gbuf[:, off * C : (off + K) * C].rearrange(
                    "p (k c) -> p k c", c=C
                )
                o1 = l1[:, hoff * C : (hoff + K2) * C].rearrange(
                    "p (k c) -> p k c", c=C
                )
                nc.vector.tensor_tensor(
                    out=o1, in0=v[:, 0:K:2, :], in1=v[:, 1:K:2, :],
                    op=mybir.AluOpType.add,
                )
                v1 = l1[:, hoff * C : (hoff + K2) * C]
                if K2 % 2 == 0 and K2 >= 4:
                    K4 = K2 // 2
                    o2 = l2[:, qoff * C : (qoff + K4) * C].rearrange(
                        "p (k c) -> p k c", c=C
                    )
                    v1r = v1.rearrange("p (k c) -> p k c", c=C)
                    nc.vector.tensor_tensor(
                        out=o2, in0=v1r[:, 0:K2:2, :], in1=v1r[:, 1:K2:2, :],
                        op=mybir.AluOpType.add,
                    )
                    tail = l2[:, qoff * C : (qoff + K4) * C].rearrange(
                        "p (k c) -> p c k", c=C
                    )
                    qoff += K4
                else:
                    tail = v1.rearrange("p (k c) -> p c k", c=C)
                nc.vector.tensor_reduce(
                    out=agg[:, ti * C : (ti + 1) * C],
                    in_=tail, axis=mybir.AxisListType.X, op=mybir.AluOpType.add,
                )
                off += K
                hoff += K2
            # add self-loop term
            nc.vector.tensor_add(
                out=agg[:, : nt * C],
                in0=agg[:, : nt * C],
                in1=xs_all[:, g0 * C : g1 * C],
            )
            # dinv scale (per node row)
            nc.vector.tensor_tensor(
                out=agg[:, : nt * C].rearrange("p (t c) -> p t c", c=C),
                in0=agg[:, : nt * C].rearrange("p (t c) -> p t c", c=C),
                in1=dinv_sh[:, g0:g1].to_broadcast([P, nt, C]),
                op=mybir.AluOpType.mult,
            )
            # batched transpose + block-diag matmul
            trps = pst.tile([P, P], f32, tag="trps")
            nc.tensor.transpose(
                out=trps[: nt * C, :], in_=agg[:, : nt * C], identity=ident[:]
            )
            aggdT = tspool.tile([P, P], f32, tag="aggdT")
            nc.scalar.copy(out=aggdT[: nt * C, :], in_=trps[: nt * C, :])
            mm = psm.tile([P, P], f32, tag="mm")
            nc.tensor.matmul(
                out=mm[:, : nt * C],
                lhsT=aggdT[: nt * C, :],
                rhs=w4a[: nt * C, : nt * C],
                start=True, stop=True,
            )
            osl = out_all[:, g0 * C : g1 * C]
            if bias_zero:
                nc.scalar.activation(
                    out=osl, in_=mm[:, : nt * C],
                    func=mybir.ActivationFunctionType.Relu,
                )
            else:
                for ti in range(nt):
                    nc.vector.tensor_add(
                        out=osl[:, ti * C : (ti + 1) * C],
                        in0=mm[:, ti * C : (ti + 1) * C],
                        in1=bsum_sb[:],
                    )
                nc.scalar.activation(
                    out=osl, in_=osl, func=mybir.ActivationFunctionType.Relu
                )

        nc.sync.dma_start(out=out_d.ap(), in_=out_all[:])

    nc.compile()
    return nc


_CACHE = {}


def _get_program(groups, SCOLS, bias_zero):
    key = (tuple(groups), SCOLS, bias_zero)
    if key not in _CACHE:
        _CACHE[key] = _build_program(groups, SCOLS, bias_zero)
    return _CACHE[key]


def run(x, edge_index, W1, b1, W2, b2, trace=False):
    prep = _host_prep(x, edge_index)
    bias_zero = not (np.any(np.asarray(b1)) or np.any(np.asarray(b2)))
    nc = _get_program(prep["groups"], prep["SCOLS"], bias_zero)

    W1 = np.ascontiguousarray(np.asarray(W1, np.float32))
    W2 = np.ascontiguousarray(np.asarray(W2, np.float32))
    b1 = np.ascontiguousarray(np.asarray(b1, np.float32))
    b2 = np.ascontiguousarray(np.asarray(b2, np.float32))

    in_maps = []
    for c in range(M):
        in_maps.append({
            "x_pad": prep["x_pad"],
            "deg_arr": prep["deg_arr"],
            "deg_shard": np.ascontiguousarray(prep["deg_shard"][c]),
            "x_shard": np.ascontiguousarray(prep["x_shard"][c].reshape(P, TILES * C)),
            "idx": np.ascontiguousarray(prep["idx_all"][c]),
            "W1": W1, "W2": W2, "b1": b1, "b2": b2,
        })

    res = run_bass_kernel_spmd(nc, in_maps, core_ids=list(range(M)), trace=trace)

    # out[p, t*C + c] = node at (tile t, partition p)
    outs = np.stack(
        [res.results[c]["out"].reshape(P, TILES, C) for c in range(M)]
    )  # [M, P, TILES, C]
    t_of = prep["pos_of"] >> 7
    p_of = prep["pos_of"] & 127
    full = outs[prep["core_of"], p_of, t_of]
    return np.ascontiguousarray(full, dtype=np.float32), res


def kernel(x, edge_index, W1, b1, W2, b2):
    out, _ = run(x, edge_index, W1, b1, W2, b2, trace=False)
    return out


# revision 10
# speedup vs baseline: 1.1898x; 1.0202x over previous
"""Trainium2 Bass kernel for nn_DiffusionGraphConv_89936615178296.

out = relu(A_hat @ (x @ (W1+W2)) + b1 + b2), A_hat = D^-1/2 (A+I) D^-1/2.

Reformulation: out = relu(dinv * (Agg(dinv*x) @ W) + b), W = W1+W2,
dinv = rsqrt(1 + in_degree); the per-edge norm dinv[src]*dinv[dst] is
separable so per-edge work is a pure gather + segment-sum of x' = dinv*x.

8 cores, no collectives. Nodes sharded round-robin in degree-sorted order
(uniform slot schedule across cores, ~13% padding). Each core:
  P0: builds the fp16 gather table x' = dinv*x for the full graph in its
      local HBM (partition-contiguous strips).
  P1: per destination tile group, gathers source rows into a [128, nt, K,
      32] fp16 slot grid (one slot column per indirect DMA; each column
      supplies one dynamic row offset per partition), pairwise-tree
      reduces on the vector engine, applies dinv, multiplies by W on the
      tensor engine (batched transpose + block-diag matmul), relu, and
      accumulates the output shard in SBUF; one final DMA writes it out.

Host does integer index prep only (degree counts, shard permutation, CSR
slot layout); all f32 arithmetic runs on device.
"""

import numpy as np
from contextlib import ExitStack

import concourse.bass as bass
import concourse.bacc as bacc
import concourse.tile as tile
from concourse import mybir
from concourse.masks import make_identity
from concourse.bass_utils import run_bass_kernel_spmd

N, E, C = 100000, 1600000, 32
M = 8
P = 128
TILES = 98                  # 98*128 = 12544 rows per core >= 12500
ROWS_PER_CORE = TILES * P
TABLE_STRIPS = 783          # 783*128 = 100224 >= N+1 (zero row at index N)
TABLE_ROWS = TABLE_STRIPS * P
ZERO_ROW = N
G_STRIP = 64                # strip width (table rows per partition per strip)
NT = 4                      # tiles per group


def _host_prep(x, edge_index):
    src = np.asarray(edge_index[0], dtype=np.int64)
    dst = np.asarray(edge_index[1], dtype=np.int64)
    x = np.asarray(x, dtype=np.float32)

    deg = np.bincount(dst, minlength=N).astype(np.int64) + 1

    order = np.argsort(-deg, kind="stable")
    ar = np.arange(N)
    core_of = np.empty(N, np.int64)
    pos_of = np.empty(N, np.int64)
    core_of[order] = ar % M
    pos_of[order] = ar // M

    degs_sorted = deg[order]
    # edges only (self-loop handled via x_shard); exact per-band max, >= 1
    Kt = [int(degs_sorted[t * P * M]) - 1 if t * P * M < N else 1 for t in range(TILES)]
    Kt = [max(1, k) for k in Kt]

    # groups of NT tiles for batched transpose/matmul; per-tile K kept tight
    groups = []  # (t0, t1, [K per tile], slot_col_offset)
    col = 0
    for g0 in range(0, TILES, NT):
        g1 = min(g0 + NT, TILES)
        groups.append((g0, g1, tuple(Kt[g0:g1]), col))
        col += sum(Kt[g0:g1])
    SCOLS = col  # total slot columns per core

    # CSR by destination
    eorder = np.argsort(dst, kind="stable")
    src_sorted = src[eorder].astype(np.int32)
    starts = np.concatenate([[0], np.cumsum(deg - 1)])

    t_of = pos_of >> 7
    p_of = pos_of & 127

    # column index of tile t's slot k
    colbase = np.empty(TILES, np.int64)
    for (g0, g1, Ks, c0) in groups:
        cc = c0
        for t in range(g0, g1):
            colbase[t] = cc
            cc += Ks[t - g0]

    idx_all = np.full((M, P, SCOLS), ZERO_ROW, np.int32)
    # edges at slots 0..deg-2 (self-loop contributed via x_shard)
    dstn = dst[eorder]
    rank = np.arange(E, dtype=np.int64) - starts[dstn]
    cols = colbase[t_of[dstn]] + rank
    idx_all[core_of[dstn], p_of[dstn], cols] = src_sorted

    # per-core own-shard features, [P, TILES, C], zero on pad rows
    x_shard = np.zeros((M, P, TILES, C), np.float32)
    x_shard[core_of, p_of, t_of] = x

    deg_pad = np.ones(TABLE_ROWS, np.float32)
    deg_pad[:N] = deg
    # strip layout: strip s covers rows [s*128*G, ...), partition p gets rows
    # base + p*Gs + g (contiguous per partition)
    deg_arr = np.ones((P, TABLE_STRIPS), np.float32)
    colpos = 0
    for s0 in range(0, TABLE_STRIPS, G_STRIP):
        g = min(G_STRIP, TABLE_STRIPS - s0)
        blk = deg_pad[s0 * P : (s0 + g) * P].reshape(P, g)
        deg_arr[:, colpos : colpos + g] = blk
        colpos += g

    deg_shard = np.ones((M, P, TILES), np.float32)
    deg_shard[core_of, p_of, t_of] = deg.astype(np.float32)

    x_pad = np.zeros((TABLE_ROWS, C), np.float32)
    x_pad[:N] = x

    return dict(
        groups=groups, SCOLS=SCOLS, idx_all=idx_all, deg_arr=deg_arr,
        deg_shard=deg_shard, x_pad=x_pad, core_of=core_of, pos_of=pos_of,
        x_shard=x_shard,
    )


def _build_program(groups, SCOLS, bias_zero):
    nc = bacc.Bacc("TRN2", target_bir_lowering=False, debug=False, num_devices=M)
    f32, f16, i32 = mybir.dt.float32, mybir.dt.float16, mybir.dt.int32

    x_d = nc.dram_tensor("x_pad", [TABLE_ROWS, C], f32, kind="ExternalInput")
    dega_d = nc.dram_tensor("deg_arr", [P, TABLE_STRIPS], f32, kind="ExternalInput")
    degs_d = nc.dram_tensor("deg_shard", [P, TILES], f32, kind="ExternalInput")
    idx_d = nc.dram_tensor("idx", [P, SCOLS], i32, kind="ExternalInput")
    xs_d = nc.dram_tensor("x_shard", [P, TILES * C], f32, kind="ExternalInput")
    w1_d = nc.dram_tensor("W1", [C, C], f32, kind="ExternalInput")
    w2_d = nc.dram_tensor("W2", [C, C], f32, kind="ExternalInput")
    b1_d = nc.dram_tensor("b1", [C], f32, kind="ExternalInput")
    b2_d = nc.dram_tensor("b2", [C], f32, kind="ExternalInput")
    table_d = nc.dram_tensor("table", [TABLE_ROWS, C], f16, kind="Internal")
    out_d = nc.dram_tensor("out", [P, TILES * C], f32, kind="ExternalOutput")

    with tile.TileContext(nc) as tc, ExitStack() as ctx:
        singles = ctx.enter_context(tc.tile_pool(name="singles", bufs=1))
        xpool = ctx.enter_context(tc.tile_pool(name="xin", bufs=3))
        tpool = ctx.enter_context(tc.tile_pool(name="tout", bufs=3))
        gpool = ctx.enter_context(tc.tile_pool(name="gather", bufs=3))
        lpool = ctx.enter_context(tc.tile_pool(name="lvl", bufs=2))
        apool = ctx.enter_context(tc.tile_pool(name="agg", bufs=3))
        tspool = ctx.enter_context(tc.tile_pool(name="trsb", bufs=3))
        pst = ctx.enter_context(tc.tile_pool(name="pst", bufs=3, space="PSUM"))
        psm = ctx.enter_context(tc.tile_pool(name="psm", bufs=3, space="PSUM"))

        # ---- singles ----
        w1_sb = singles.tile([C, C], f32)
        w2_sb = singles.tile([C, C], f32)
        nc.sync.dma_start(out=w1_sb[:], in_=w1_d.ap())
        nc.sync.dma_start(out=w2_sb[:], in_=w2_d.ap())

        # block-diagonal W4 [128, 128]: NT copies of Wsum on the diagonal
        w4a = singles.tile([P, P], f32)
        w4b = singles.tile([P, P], f32)
        nc.vector.memset(w4a[:], 0.0)
        nc.gpsimd.memset(w4b[:], 0.0)
        for t in range(NT):
            sl = slice(t * C, (t + 1) * C)
            nc.sync.dma_start(out=w4a[sl, sl], in_=w1_d.ap())
            nc.sync.dma_start(out=w4b[sl, sl], in_=w2_d.ap())
        nc.vector.tensor_add(out=w4a[:], in0=w4a[:], in1=w4b[:])

        if not bias_zero:
            b1_sb = singles.tile([P, C], f32)
            b2_sb = singles.tile([P, C], f32)
            nc.sync.dma_start(
                out=b1_sb[:], in_=bass.AP(tensor=b1_d, offset=0, ap=[[0, P], [1, C]])
            )
            nc.sync.dma_start(
                out=b2_sb[:], in_=bass.AP(tensor=b2_d, offset=0, ap=[[0, P], [1, C]])
            )
            bsum_sb = singles.tile([P, C], f32)
            nc.vector.tensor_add(out=bsum_sb[:], in0=b1_sb[:], in1=b2_sb[:])

        ident = singles.tile([P, P], f32)
        make_identity(nc, ident[:])

        idx_sb = singles.tile([P, SCOLS], i32)
        nc.sync.dma_start(out=idx_sb[:], in_=idx_d.ap())

        dinv_all = singles.tile([P, TABLE_STRIPS], f32)
        nc.sync.dma_start(out=dinv_all[:], in_=dega_d.ap())
        nc.scalar.sqrt(out=dinv_all[:], in_=dinv_all[:])
        nc.vector.reciprocal(out=dinv_all[:], in_=dinv_all[:])

        dinv_sh = singles.tile([P, TILES], f32)
        nc.sync.dma_start(out=dinv_sh[:], in_=degs_d.ap())
        nc.scalar.sqrt(out=dinv_sh[:], in_=dinv_sh[:])
        nc.vector.reciprocal(out=dinv_sh[:], in_=dinv_sh[:])

        out_all = singles.tile([P, TILES * C], f32)

        # self-loop contribution x' = dinv * x for own shard rows
        xs_all = singles.tile([P, TILES * C], f32)
        nc.sync.dma_start(out=xs_all[:], in_=xs_d.ap())
        nc.vector.tensor_tensor(
            out=xs_all[:].rearrange("p (t c) -> p t c", c=C),
            in0=xs_all[:].rearrange("p (t c) -> p t c", c=C),
            in1=dinv_sh[:].to_broadcast([P, TILES, C]),
            op=mybir.AluOpType.mult,
        )

        # ---- P0: table build, partition-contiguous strips ----
        colpos = 0
        si = 0
        for s0 in range(0, TABLE_STRIPS, G_STRIP):
            g = min(G_STRIP, TABLE_STRIPS - s0)
            # rows s0*128 + p*g + j  for j in [0, g)
            x_ap = bass.AP(
                tensor=x_d, offset=s0 * P * C,
                ap=[[g * C, P], [C, g], [1, C]],
            )
            t_ap = bass.AP(
                tensor=table_d, offset=s0 * P * C,
                ap=[[g * C, P], [C, g], [1, C]],
            )
            x_sb = xpool.tile([P, G_STRIP, C], f32, tag="x")
            nc.sync.dma_start(out=x_sb[:, :g, :], in_=x_ap)
            t_sb = tpool.tile([P, G_STRIP, C], f16, tag="t")
            nc.vector.tensor_tensor(
                out=t_sb[:, :g, :],
                in0=x_sb[:, :g, :],
                in1=dinv_all[:, colpos : colpos + g].to_broadcast([P, g, C]),
                op=mybir.AluOpType.mult,
            )
            nc.sync.dma_start(out=t_ap, in_=t_sb[:, :g, :])
            colpos += g
            si += 1

        # ---- P1 ----
        max_sg = max(sum(Ks) for _, _, Ks, _ in groups)
        for (g0, g1, Ks, c0) in groups:
            nt = g1 - g0
            sg = sum(Ks)
            # gather: one indirect DMA per slot column
            gbuf = gpool.tile([P, max_sg * C], f16, tag="gbuf")
            for j in range(sg):
                nc.gpsimd.indirect_dma_start(
                    out=gbuf[:, j * C : (j + 1) * C],
                    out_offset=None,
                    in_=table_d.ap(),
                    in_offset=bass.IndirectOffsetOnAxis(
                        ap=idx_sb[:, c0 + j : c0 + j + 1], axis=0
                    ),
                )
            # per-tile pairwise tree: L1, L2 (fp16), then strided f32 tail
            l1 = lpool.tile([P, (max_sg // 2 + NT) * C], f16, tag="l1")
            l2 = lpool.tile([P, (max_sg // 4) * C], f16, tag="l2")
            agg = apool.tile([P, NT * C], f32, tag="agg")
            off = 0   # slot-column offset within gbuf
            hoff = 0  # half-column offset within l1
            for ti in range(nt):
                K = Ks[ti]
                H = K // 2
                R = H + (K % 2)
                v = gbuf[:, off * C : (off + K) * C].rearrange(
                    "p (k c) -> p k c", c=C
                )
                if H > 0:
                    o1 = l1[:, hoff * C : (hoff + H) * C].rearrange(
                        "p (k c) -> p k c", c=C
                    )
                    nc.vector.tensor_tensor(
                        out=o1, in0=v[:, 0 : 2 * H : 2, :], in1=v[:, 1 : 2 * H : 2, :],
                        op=mybir.AluOpType.add,
                    )
                if K % 2 == 1:
                    nc.vector.tensor_copy(
                        out=l1[:, (hoff + H) * C : (hoff + R) * C],
                        in_=gbuf[:, (off + K - 1) * C : (off + K) * C],
                    )
                tail = l1[:, hoff * C : (hoff + R) * C].rearrange(
                    "p (k c) -> p c k", c=C
                )
                nc.vector.tensor_reduce(
                    out=agg[:, ti * C : (ti + 1) * C],
                    in_=tail, axis=mybir.AxisListType.X, op=mybir.AluOpType.add,
                )
                off += K
                hoff += R
            # add self-loop term
            nc.vector.tensor_add(
                out=agg[:, : nt * C],
                in0=agg[:, : nt * C],
                in1=xs_all[:, g0 * C : g1 * C],
            )
            # dinv scale (per node row)
            nc.vector.tensor_tensor(
                out=agg[:, : nt * C].rearrange("p (t c) -> p t c", c=C),
                in0=agg[:, : nt * C].rearrange("p (t c) -> p t c", c=C),
                in1=dinv_sh[:, g0:g1].to_broadcast([P, nt, C]),
                op=mybir.AluOpType.mult,
            )
            # batched transpose + block-diag matmul
            trps = pst.tile([P, P], f32, tag="trps")
            nc.tensor.transpose(
                out=trps[: nt * C, :], in_=agg[:, : nt * C], identity=ident[:]
            )
            aggdT = tspool.tile([P, P], f32, tag="aggdT")
            nc.scalar.copy(out=aggdT[: nt * C, :], in_=trps[: nt * C, :])
            mm = psm.tile([P, P], f32, tag="mm")
            nc.tensor.matmul(
                out=mm[:, : nt * C],
                lhsT=aggdT[: nt * C, :],
                rhs=w4a[: nt * C, : nt * C],
                start=True, stop=True,
            )
            osl = out_all[:, g0 * C : g1 * C]
            if bias_zero:
                nc.scalar.activation(
                    out=osl, in_=mm[:, : nt * C],
                    func=mybir.ActivationFunctionType.Relu,
                )
            else:
                for ti in range(nt):
                    nc.vector.tensor_add(
                        out=osl[:, ti * C : (ti + 1) * C],
                        in0=mm[:, ti * C : (ti + 1) * C],
                        in1=bsum_sb[:],
                    )
                nc.scalar.activation(
                    out=osl, in_=osl, func=mybir.ActivationFunctionType.Relu
                )

        nc.sync.dma_start(out=out_d.ap(), in_=out_all[:])

    nc.compile()
    return nc


_CACHE = {}


def _get_program(groups, SCOLS, bias_zero):
    key = (tuple(groups), SCOLS, bias_zero)
    if key not in _CACHE:
        _CACHE[key] = _build_program(groups, SCOLS, bias_zero)
    return _CACHE[key]


def run(x, edge_index, W1, b1, W2, b2, trace=False):
    prep = _host_prep(x, edge_index)
    bias_zero = not (np.any(np.asarray(b1)) or np.any(np.asarray(b2)))
    nc = _get_program(prep["groups"], prep["SCOLS"], bias_zero)

    W1 = np.ascontiguousarray(np.asarray(W1, np.float32))
    W2 = np.ascontiguousarray(np.asarray(W2, np.float32))
    b1 = np.ascontiguousarray(np.asarray(b1, np.float32))
    b2 = np.ascontiguousarray(np.asarray(b2, np.float32))

    in_maps = []
    for c in range(M):
        in_maps.append({
            "x_pad": prep["x_pad"],
            "deg_arr": prep["deg_arr"],
            "deg_shard": np.ascontiguousarray(prep["deg_shard"][c]),
            "x_shard": np.ascontiguousarray(prep["x_shard"][c].reshape(P, TILES * C)),
            "idx": np.ascontiguousarray(prep["idx_all"][c]),
            "W1": W1, "W2": W2, "b1": b1, "b2": b2,
        })

    res = run_bass_kernel_spmd(nc, in_maps, core_ids=list(range(M)), trace=trace)

    # out[p, t*C + c] = node at (tile t, partition p)
    outs = np.stack(
        [res.results[c]["out"].reshape(P, TILES, C) for c in range(M)]
    )  # [M, P, TILES, C]
    t_of = prep["pos_of"] >> 7
    p_of = prep["pos_of"] & 127
    full = outs[prep["core_of"], p_of, t_of]
    return np.ascontiguousarray(full, dtype=np.float32), res


def kernel(x, edge_index, W1, b1, W2, b2):
    out, _ = run(x, edge_index, W1, b1, W2, b2, trace=False)
    return out
